# revision 4
# baseline (speedup 1.0000x reference)
"""Trainium2 Bass kernel for DynamicSpatialAttention.

reference semantics (per sample b):
  pooled = x.mean((2,3))                       [C]
  z      = relu(pooled @ w1 + b1)              [C]
  kern   = (z @ w2 + b2).reshape(3,3)          per-sample 3x3 kernel
  m      = x.mean(1)                           [H,W]   channel-mean map
  att    = sigmoid(conv2d(m, kern, pad=1))     [H,W]
  out    = x * att[None]

Distribution: data-parallel over batch B across 8 NeuronCores (4 samples
per core, fully independent -- no collectives).

The problem is pure memory streaming (read x once, write out once); the
2e-2 tolerance leaves a lot of precision headroom, so x is landed in
SBUF as bf16 (SWDGE cast during the load DMA) and out is written as
bf16 (host upcasts to f32), cutting HBM traffic to ~100 MB/core.

HBM pays a read/write turnaround penalty when load and store packets
interleave (measured ~300 GB/s mixed vs ~354 GB/s single-direction), so
ALL bulk traffic rides the single gpsimd SWDGE queue, software-
pipelined: sample b's loads are emitted BEFORE sample b-1's stores, so
the FIFO alternates pure-read and pure-write phases at full rate.  The
attention map, broadcasts and in-place multiplies for sample b-1 are
computed while sample b loads, so its store descriptors are already
eligible the moment the load phase drains.

Engine budget is dictated by DVE perf modes (tensor_scalar+accum_out
only has a 1x uop; tensor_tensor is 2x only if every operand is bf16 in
SBUF): pooled partial sums are split between VectorE and ScalarE
(accum_out side-sums of dummy copies), the broadcast s tiles are staged
PSUM->SBUF bf16 by ScalarE, and VectorE runs the in-place x *= s
multiplies at the 2x all-bf16 rate.  TensorE accumulates the channel-
sum map with shifted one-hot matmuls as chunks land, runs the tiny
kernel-generator matmuls, the 3x3 conv as three banded matmuls (bands =
vertical taps, PSUM column offsets = horizontal taps), and the K=1
ones-matmuls that broadcast s across partitions.
"""

import numpy as np

B, C, H, W = 32, 256, 128, 128
HW = H * W
KS = 3
N_CORES = 8
BS = B // N_CORES


def build_nc(bs=BS, c=C, h=H, w=W):
    import concourse.bass as bass  # noqa: F401
    import concourse.tile as tile
    from concourse import bacc, mybir
    from concourse.masks import make_identity

    f32 = mybir.dt.float32
    bf16 = mybir.dt.bfloat16
    AX = mybir.AxisListType
    AF = mybir.ActivationFunctionType

    hw = h * w
    assert c == 256, "kernel assumes 2 channel halves of 128"
    QW = 512                      # msum free dim / broadcast-matmul width
    NQ = hw // QW                 # number of 512-wide hw chunks (rows of msum)
    assert NQ <= 32
    CH = 2048                     # load/store chunk free width
    NCH = hw // CH                # chunks per sample-half
    QPC = CH // QW                # 512-chunks per load chunk
    MW = 1024                     # multiply/bps granularity
    NM = hw // MW                 # multiply segments per half
    SRW = hw // 2                 # staged-s cols per partition (rows 0 and 64)
    QPR = SRW // QW               # 512-chunks per staged-s partition

    nc = bacc.Bacc("TRN2", target_bir_lowering=False, debug=False)
    x_d = nc.declare_dram_parameter("x", [bs, c, hw], f32, isOutput=False)
    w1_d = nc.declare_dram_parameter("w1", [c, c], f32, isOutput=False)
    b1_d = nc.declare_dram_parameter("b1", [c], f32, isOutput=False)
    w2_d = nc.declare_dram_parameter("w2", [c, KS * KS], f32, isOutput=False)
    b2_d = nc.declare_dram_parameter("b2", [KS * KS], f32, isOutput=False)
    out_d = nc.declare_dram_parameter("out", [bs, c, hw], bf16, isOutput=True)

    with tile.TileContext(nc) as tc:
        with (
            tc.tile_pool(name="xr", bufs=5) as xr,
            tc.tile_pool(name="scs", bufs=1) as scs,
            tc.tile_pool(name="srp", bufs=1) as srp,
            tc.tile_pool(name="bsp", bufs=4) as bsp,
            tc.tile_pool(name="small", bufs=2) as small,
            tc.tile_pool(name="singles", bufs=1) as singles,
            tc.tile_pool(name="convt", bufs=2) as convt,
            tc.tile_pool(name="pm", bufs=3, space="PSUM") as pm,
            tc.tile_pool(name="pb", bufs=2, space="PSUM") as pb,
            tc.tile_pool(name="ps", bufs=1, space="PSUM") as ps,
        ):
            # ---- constants / weights (loaded once) ----
            estrip = singles.tile([128, 2 * NQ], bf16)
            nc.vector.memset(estrip, 0.0)
            nc.vector.memset(estrip[:, NQ : NQ + 1], 1.0)
            ones_r = singles.tile([128, 128], bf16)
            nc.vector.memset(ones_r, 1.0)
            ones_rf = singles.tile([1, 128], f32)
            nc.vector.memset(ones_rf, 1.0)
            # 0/1 diagonal masks used to build the banded conv matrices:
            # ident[h,h']=d(h'=h), d_up[h,:]=e_{h+1}, d_dn[h,:]=e_{h-1}
            ident = singles.tile([h, h], bf16)
            make_identity(nc, ident)
            d_up = singles.tile([h, h], bf16)
            d_dn = singles.tile([h, h], bf16)
            nc.vector.memset(d_up, 0.0)
            nc.vector.memset(d_dn, 0.0)
            nc.gpsimd.dma_start(out=d_up[0 : h - 1, :], in_=ident[1:h, :])
            nc.gpsimd.dma_start(out=d_dn[1:h, :], in_=ident[0 : h - 1, :])
            w1_sb = singles.tile([128, 2, c], f32)  # [i_part, i_blk, j]
            nc.sync.dma_start(
                out=w1_sb, in_=w1_d.rearrange("(ib i) j -> i ib j", ib=2)
            )
            # fold the 1/HW of the spatial mean into w1 so pooled can stay
            # a raw sum (one chain hop less per sample)
            nc.scalar.activation(
                out=w1_sb, in_=w1_sb, func=AF.Copy, scale=1.0 / hw
            )
            w2_sb = singles.tile([128, 2, KS * KS], f32)  # [j_part, j_blk, t]
            nc.sync.dma_start(
                out=w2_sb, in_=w2_d.rearrange("(jb j) t -> j jb t", jb=2)
            )
            b1_sb = singles.tile([128, 2], f32)
            nc.sync.dma_start(
                out=b1_sb, in_=b1_d.rearrange("(jb j) -> j jb", jb=2)
            )
            b2_sb = singles.tile([1, KS * KS], f32)
            nc.sync.dma_start(
                out=b2_sb, in_=b2_d.rearrange("(o t) -> o t", o=1)
            )

            def emit_loads(b):
                """Loads (gpsimd q0, f32->bf16 cast) + chansum matmuls +
                pooled partial accumulation as chunks land."""
                msum = pm.tile([NQ, QW], f32, tag="msum", name="msum")
                parts = small.tile([128, 16], f32, tag="parts")
                xres = []
                i_mm = 0
                n_mm = 2 * NCH * QPC
                for hh in range(2):
                    t = xr.tile([128, hw], bf16, tag="xres", name="xres")
                    xres.append(t)
                    for q in range(NCH):
                        nc.gpsimd.dma_start(
                            out=t[:, CH * q : CH * (q + 1)],
                            in_=x_d[
                                b, 128 * hh : 128 * (hh + 1), CH * q : CH * (q + 1)
                            ],
                        )
                        # pooled partial sums in 2048-wide blocks (bounded
                        # head-of-line blocking of chain ops on the same
                        # engine); h0 on VectorE reduce_sum, h1 on ScalarE
                        # accum_out side-sums of a dummy copy
                        for s2 in range(CH // 2048):
                            seg = t[
                                :, CH * q + 2048 * s2 : CH * q + 2048 * (s2 + 1)
                            ]
                            pi = 8 * hh + (CH // 2048) * q + s2
                            if hh == 0:
                                nc.vector.reduce_sum(
                                    out=parts[:, pi : pi + 1], in_=seg, axis=AX.X
                                )
                            else:
                                sc = scs.tile(
                                    [128, 2048], bf16, tag="scs", name="scs"
                                )
                                nc.scalar.activation(
                                    out=sc,
                                    in_=seg,
                                    func=AF.Copy,
                                    accum_out=parts[:, pi : pi + 1],
                                )
                        for s in range(QPC):
                            Q = QPC * q + s
                            nc.tensor.matmul(
                                msum,
                                estrip[:, NQ - Q : 2 * NQ - Q],
                                t[:, QW * Q : QW * (Q + 1)],
                                start=(i_mm == 0),
                                stop=(i_mm == n_mm - 1),
                            )
                            i_mm += 1
                return msum, parts, xres

            def emit_chain(msum, parts):
                """pooled -> z -> kern -> banded 3x3 conv -> sigmoid ->
                staged s (partitions 0/64)."""
                pooled = small.tile([128, 2], f32, tag="pooled")
                nc.vector.reduce_sum(
                    out=pooled[:, 0:1], in_=parts[:, 0:8], axis=AX.X
                )
                nc.vector.reduce_sum(
                    out=pooled[:, 1:2], in_=parts[:, 8:16], axis=AX.X
                )
                z_sb = small.tile([128, 2], f32, tag="z")
                for j in range(2):
                    zp = ps.tile([128, 1], f32, tag="zsmall", name="zp")
                    for i in range(2):
                        nc.tensor.matmul(
                            zp,
                            w1_sb[:, i, 128 * j : 128 * (j + 1)],
                            pooled[:, i : i + 1],
                            start=(i == 0),
                            stop=(i == 1),
                        )
                    nc.scalar.activation(
                        out=z_sb[:, j : j + 1],
                        in_=zp,
                        func=AF.Relu,
                        bias=b1_sb[:, j : j + 1],
                        scale=1.0,
                    )
                kp = ps.tile([1, KS * KS], f32, tag="zsmall", name="kp")
                for j in range(2):
                    nc.tensor.matmul(
                        kp,
                        z_sb[:, j : j + 1],
                        w2_sb[:, j, :],
                        start=(j == 0),
                        stop=(j == 1),
                    )
                kern = small.tile([1, KS * KS], f32, tag="kern")
                nc.vector.tensor_add(out=kern, in0=kp, in1=b2_sb)
                kbp = ps.tile([128, KS * KS], f32, tag="zsmall", name="kbp")
                nc.tensor.matmul(kbp, ones_rf, kern, start=True, stop=True)
                kb = small.tile([128, KS * KS], f32, tag="kb")
                # fold the 1/C of the channel mean into the conv weights
                nc.scalar.activation(out=kb, in_=kbp, func=AF.Copy, scale=1.0 / c)

                # conv2d(m, kern) as 3 banded matmuls: for each kernel
                # column dx, T_dx[h,h'] = k[h-h'+1, dx] is tridiagonal;
                # att[:, w-shifted] += T_dx.T @ m[:, w-shifted].  Vertical
                # padding is implicit in the band clipping, horizontal
                # padding in the PSUM column offsets.
                m32 = small.tile([NQ, QW], bf16, tag="m32")
                nc.scalar.copy(out=m32, in_=msum)
                m_sq = convt.tile([h, w], bf16, tag="msq")
                nc.sync.dma_start(out=m_sq, in_=m32)
                tb = convt.tile([h, h], bf16, tag="tb")
                t_mats = []
                for dx in range(3):
                    T = convt.tile([h, h], bf16, tag=f"T{dx}", name="T")
                    nc.vector.tensor_scalar_mul(
                        out=T, in0=ident, scalar1=kb[:h, 3 + dx : 4 + dx]
                    )
                    nc.vector.tensor_scalar_mul(
                        out=tb, in0=d_up, scalar1=kb[:h, dx : dx + 1]
                    )
                    nc.vector.tensor_add(out=T, in0=T, in1=tb)
                    nc.vector.tensor_scalar_mul(
                        out=tb, in0=d_dn, scalar1=kb[:h, 6 + dx : 7 + dx]
                    )
                    nc.vector.tensor_add(out=T, in0=T, in1=tb)
                    t_mats.append(T)
                attp = pm.tile([h, w], f32, tag="msum", name="attp")
                nc.tensor.matmul(attp, t_mats[1], m_sq, start=True, stop=False)
                nc.tensor.matmul(
                    attp[:, 0 : w - 1],
                    t_mats[2],
                    m_sq[:, 1:w],
                    start=False,
                    stop=False,
                )
                nc.tensor.matmul(
                    attp[:, 1:w],
                    t_mats[0],
                    m_sq[:, 0 : w - 1],
                    start=False,
                    stop=True,
                )
                s_bf = convt.tile([h, w], bf16, tag="sbf")
                nc.scalar.activation(out=s_bf, in_=attp, func=AF.Sigmoid)
                # stage s onto partitions 0/64 (legal matmul base
                # partitions) so the K=1 broadcast matmuls can read it
                sr = srp.tile([128, SRW], bf16, tag="sr", name="sr")
                nc.sync.dma_start(out=sr[0:128:64, :], in_=s_bf)
                return sr

            def emit_mults(sr, xres):
                """Broadcast s via K=1 matmuls, stage PSUM->SBUF bf16 on
                ScalarE, multiply x in place on VectorE (2x all-bf16)."""
                for m in range(NM):
                    bp = pb.tile([128, MW], f32, tag="bp", name="bp")
                    for s in range(MW // QW):
                        Q = (MW // QW) * m + s
                        r = 64 * (Q // QPR)
                        nc.tensor.matmul(
                            bp[:, QW * s : QW * (s + 1)],
                            ones_r[r : r + 1, :],
                            sr[r : r + 1, QW * (Q % QPR) : QW * (Q % QPR + 1)],
                            start=True,
                            stop=True,
                        )
                    bps = bsp.tile([128, MW], bf16, tag="bps", name="bps")
                    nc.scalar.copy(out=bps, in_=bp)
                    for hh in range(2):
                        nc.vector.tensor_mul(
                            out=xres[hh][:, MW * m : MW * (m + 1)],
                            in0=xres[hh][:, MW * m : MW * (m + 1)],
                            in1=bps,
                        )

            def emit_stores(b, xres):
                """Stores (bf16) on the two HWDGE rings, concurrent with
                the next sample's loads on the SWDGE queue.  The last
                sample has no loads behind it, so its stores also use the
                idle SWDGE queue as a third ring."""
                engs = (
                    [nc.scalar, nc.sync, nc.gpsimd]
                    if b == bs - 1
                    else [nc.scalar, nc.sync]
                )
                for q in range(NCH):
                    for hh in range(2):
                        out_eng = engs[(2 * q + hh) % len(engs)]
                        out_eng.dma_start(
                            out=out_d[
                                b, 128 * hh : 128 * (hh + 1), CH * q : CH * (q + 1)
                            ],
                            in_=xres[hh][:, CH * q : CH * (q + 1)],
                        )

            for b in range(bs):
                msum, parts, xres = emit_loads(b)
                # chain ops are latency-critical (they gate the whole
                # multiply+store phase); keep the static scheduler from
                # burying them behind ready bulk work
                with tc.high_priority():
                    sr = emit_chain(msum, parts)
                emit_mults(sr, xres)
                emit_stores(b, xres)

    nc.finalize()
    return nc


_NC_CACHE = {}


def _get_nc(key=(BS, C, H, W)):
    if key not in _NC_CACHE:
        _NC_CACHE[key] = build_nc(*key)
    return _NC_CACHE[key]


def kernel(x, w1, b1, w2, b2):
    from concourse.bass_utils import run_bass_kernel_spmd

    x = np.ascontiguousarray(x, dtype=np.float32)
    nc = _get_nc()
    in_maps = []
    for i in range(N_CORES):
        in_maps.append(
            {
                "x": x[i * BS : (i + 1) * BS].reshape(BS, C, HW),
                "w1": np.ascontiguousarray(w1, dtype=np.float32),
                "b1": np.ascontiguousarray(b1, dtype=np.float32),
                "w2": np.ascontiguousarray(w2, dtype=np.float32),
                "b2": np.ascontiguousarray(b2, dtype=np.float32),
            }
        )
    res = run_bass_kernel_spmd(nc, in_maps, list(range(N_CORES)))
    out = np.concatenate(
        [
            np.asarray(r["out"]).astype(np.float32).reshape(BS, C, H, W)
            for r in res.results
        ],
        axis=0,
    )
    return out


# revision 6
# speedup vs baseline: 1.0157x; 1.0157x over previous
"""Trainium2 Bass kernel for DynamicSpatialAttention.

reference semantics (per sample b):
  pooled = x.mean((2,3))                       [C]
  z      = relu(pooled @ w1 + b1)              [C]
  kern   = (z @ w2 + b2).reshape(3,3)          per-sample 3x3 kernel
  m      = x.mean(1)                           [H,W]   channel-mean map
  att    = sigmoid(conv2d(m, kern, pad=1))     [H,W]
  out    = x * att[None]

Distribution: data-parallel over batch B across 8 NeuronCores (4 samples
per core, fully independent -- no collectives).

The problem is pure memory streaming (read x once, write out once); the
2e-2 tolerance leaves a lot of precision headroom, so x is landed in
SBUF as bf16 (SWDGE cast during the load DMA) and out is written as
bf16 (host upcasts to f32), cutting HBM traffic to ~100 MB/core.

HBM pays a read/write turnaround penalty when load and store packets
interleave (measured ~300 GB/s mixed vs ~354 GB/s single-direction), so
ALL bulk traffic rides the single gpsimd SWDGE queue, software-
pipelined: sample b's loads are emitted BEFORE sample b-1's stores, so
the FIFO alternates pure-read and pure-write phases at full rate.  The
attention map, broadcasts and in-place multiplies for sample b-1 are
computed while sample b loads, so its store descriptors are already
eligible the moment the load phase drains.

Engine budget is dictated by DVE perf modes (tensor_scalar+accum_out
only has a 1x uop; tensor_tensor is 2x only if every operand is bf16 in
SBUF): pooled partial sums are split between VectorE and ScalarE
(accum_out side-sums of dummy copies), the broadcast s tiles are staged
PSUM->SBUF bf16 by ScalarE, and VectorE runs the in-place x *= s
multiplies at the 2x all-bf16 rate.  TensorE accumulates the channel-
sum map with shifted one-hot matmuls as chunks land, runs the tiny
kernel-generator matmuls, the 3x3 conv as three banded matmuls (bands =
vertical taps, PSUM column offsets = horizontal taps), and the K=1
ones-matmuls that broadcast s across partitions.
"""

import numpy as np

B, C, H, W = 32, 256, 128, 128
HW = H * W
KS = 3
N_CORES = 8
BS = B // N_CORES


def build_nc(bs=BS, c=C, h=H, w=W):
    import concourse.bass as bass  # noqa: F401
    import concourse.tile as tile
    from concourse import bacc, mybir
    from concourse.masks import make_identity

    f32 = mybir.dt.float32
    bf16 = mybir.dt.bfloat16
    AX = mybir.AxisListType
    AF = mybir.ActivationFunctionType

    hw = h * w
    assert c == 256, "kernel assumes 2 channel halves of 128"
    QW = 512                      # msum free dim / broadcast-matmul width
    NQ = hw // QW                 # number of 512-wide hw chunks (rows of msum)
    assert NQ <= 32
    CH = 2048                     # load/store chunk free width
    NCH = hw // CH                # chunks per sample-half
    QPC = CH // QW                # 512-chunks per load chunk
    MW = 1024                     # multiply/bps granularity
    NM = hw // MW                 # multiply segments per half
    SRW = hw // 2                 # staged-s cols per partition (rows 0 and 64)
    QPR = SRW // QW               # 512-chunks per staged-s partition

    nc = bacc.Bacc("TRN2", target_bir_lowering=False, debug=False)
    x_d = nc.declare_dram_parameter("x", [bs, c, hw], f32, isOutput=False)
    w1_d = nc.declare_dram_parameter("w1", [c, c], f32, isOutput=False)
    b1_d = nc.declare_dram_parameter("b1", [c], f32, isOutput=False)
    w2_d = nc.declare_dram_parameter("w2", [c, KS * KS], f32, isOutput=False)
    b2_d = nc.declare_dram_parameter("b2", [KS * KS], f32, isOutput=False)
    out_d = nc.declare_dram_parameter("out", [bs, c, hw], bf16, isOutput=True)

    with tile.TileContext(nc) as tc:
        with (
            tc.tile_pool(name="xr", bufs=5) as xr,
            tc.tile_pool(name="scs", bufs=1) as scs,
            tc.tile_pool(name="srp", bufs=1) as srp,
            tc.tile_pool(name="bsp", bufs=4) as bsp,
            tc.tile_pool(name="small", bufs=2) as small,
            tc.tile_pool(name="singles", bufs=1) as singles,
            tc.tile_pool(name="convt", bufs=2) as convt,
            tc.tile_pool(name="pm", bufs=3, space="PSUM") as pm,
            tc.tile_pool(name="pb", bufs=2, space="PSUM") as pb,
            tc.tile_pool(name="ps", bufs=1, space="PSUM") as ps,
        ):
            # ---- constants / weights (loaded once) ----
            estrip = singles.tile([128, 2 * NQ], bf16)
            nc.vector.memset(estrip, 0.0)
            nc.vector.memset(estrip[:, NQ : NQ + 1], 1.0)
            ones_r = singles.tile([128, 128], bf16)
            nc.vector.memset(ones_r, 1.0)
            ones_rf = singles.tile([1, 128], f32)
            nc.vector.memset(ones_rf, 1.0)
            # 0/1 diagonal masks used to build the banded conv matrices:
            # ident[h,h']=d(h'=h), d_up[h,:]=e_{h+1}, d_dn[h,:]=e_{h-1}
            ident = singles.tile([h, h], bf16)
            make_identity(nc, ident)
            d_up = singles.tile([h, h], bf16)
            d_dn = singles.tile([h, h], bf16)
            nc.vector.memset(d_up, 0.0)
            nc.vector.memset(d_dn, 0.0)
            # sync ring, NOT gpsimd: these wait on make_identity's memsets,
            # and at the head of the gpsimd queue they would delay every
            # sample-0 load descriptor behind them (~6us of HBM idle)
            nc.sync.dma_start(out=d_up[0 : h - 1, :], in_=ident[1:h, :])
            nc.sync.dma_start(out=d_dn[1:h, :], in_=ident[0 : h - 1, :])
            w1_sb = singles.tile([128, 2, c], f32)  # [i_part, i_blk, j]
            nc.sync.dma_start(
                out=w1_sb, in_=w1_d.rearrange("(ib i) j -> i ib j", ib=2)
            )
            # fold the 1/HW of the spatial mean into w1 so pooled can stay
            # a raw sum (one chain hop less per sample)
            nc.scalar.activation(
                out=w1_sb, in_=w1_sb, func=AF.Copy, scale=1.0 / hw
            )
            w2_sb = singles.tile([128, 2, KS * KS], f32)  # [j_part, j_blk, t]
            nc.sync.dma_start(
                out=w2_sb, in_=w2_d.rearrange("(jb j) t -> j jb t", jb=2)
            )
            b1_sb = singles.tile([128, 2], f32)
            nc.sync.dma_start(
                out=b1_sb, in_=b1_d.rearrange("(jb j) -> j jb", jb=2)
            )
            b2_sb = singles.tile([1, KS * KS], f32)
            nc.sync.dma_start(
                out=b2_sb, in_=b2_d.rearrange("(o t) -> o t", o=1)
            )

            def emit_loads(b):
                """Loads (gpsimd q0, f32->bf16 cast) + chansum matmuls +
                pooled partial accumulation as chunks land."""
                msum = pm.tile([NQ, QW], f32, tag="msum", name="msum")
                parts = small.tile([128, 16], f32, tag="parts")
                xres = []
                i_mm = 0
                n_mm = 2 * NCH * QPC
                for hh in range(2):
                    t = xr.tile([128, hw], bf16, tag="xres", name="xres")
                    xres.append(t)
                    for q in range(NCH):
                        nc.gpsimd.dma_start(
                            out=t[:, CH * q : CH * (q + 1)],
                            in_=x_d[
                                b, 128 * hh : 128 * (hh + 1), CH * q : CH * (q + 1)
                            ],
                        )
                        # pooled partial sums in 2048-wide blocks (bounded
                        # head-of-line blocking of chain ops on the same
                        # engine); h0 on VectorE reduce_sum, h1 on ScalarE
                        # accum_out side-sums of a dummy copy
                        for s2 in range(CH // 2048):
                            seg = t[
                                :, CH * q + 2048 * s2 : CH * q + 2048 * (s2 + 1)
                            ]
                            pi = 8 * hh + (CH // 2048) * q + s2
                            if hh == 0:
                                nc.vector.reduce_sum(
                                    out=parts[:, pi : pi + 1], in_=seg, axis=AX.X
                                )
                            else:
                                sc = scs.tile(
                                    [128, 2048], bf16, tag="scs", name="scs"
                                )
                                nc.scalar.activation(
                                    out=sc,
                                    in_=seg,
                                    func=AF.Copy,
                                    accum_out=parts[:, pi : pi + 1],
                                )
                        for s in range(QPC):
                            Q = QPC * q + s
                            nc.tensor.matmul(
                                msum,
                                estrip[:, NQ - Q : 2 * NQ - Q],
                                t[:, QW * Q : QW * (Q + 1)],
                                start=(i_mm == 0),
                                stop=(i_mm == n_mm - 1),
                            )
                            i_mm += 1
                return msum, parts, xres

            def emit_chain(msum, parts):
                """pooled -> z -> kern -> banded 3x3 conv -> sigmoid ->
                staged s (partitions 0/64)."""
                pooled = small.tile([128, 2], f32, tag="pooled")
                nc.vector.reduce_sum(
                    out=pooled[:, 0:1], in_=parts[:, 0:8], axis=AX.X
                )
                nc.vector.reduce_sum(
                    out=pooled[:, 1:2], in_=parts[:, 8:16], axis=AX.X
                )
                z_sb = small.tile([128, 2], f32, tag="z")
                for j in range(2):
                    zp = ps.tile([128, 1], f32, tag="zsmall", name="zp")
                    for i in range(2):
                        nc.tensor.matmul(
                            zp,
                            w1_sb[:, i, 128 * j : 128 * (j + 1)],
                            pooled[:, i : i + 1],
                            start=(i == 0),
                            stop=(i == 1),
                        )
                    nc.scalar.activation(
                        out=z_sb[:, j : j + 1],
                        in_=zp,
                        func=AF.Relu,
                        bias=b1_sb[:, j : j + 1],
                        scale=1.0,
                    )
                kp = ps.tile([1, KS * KS], f32, tag="zsmall", name="kp")
                for j in range(2):
                    nc.tensor.matmul(
                        kp,
                        z_sb[:, j : j + 1],
                        w2_sb[:, j, :],
                        start=(j == 0),
                        stop=(j == 1),
                    )
                kern = small.tile([1, KS * KS], f32, tag="kern")
                nc.vector.tensor_add(out=kern, in0=kp, in1=b2_sb)
                kbp = ps.tile([128, KS * KS], f32, tag="zsmall", name="kbp")
                nc.tensor.matmul(kbp, ones_rf, kern, start=True, stop=True)
                kb = small.tile([128, KS * KS], f32, tag="kb")
                # fold the 1/C of the channel mean into the conv weights
                nc.scalar.activation(out=kb, in_=kbp, func=AF.Copy, scale=1.0 / c)

                # conv2d(m, kern) as 3 banded matmuls: for each kernel
                # column dx, T_dx[h,h'] = k[h-h'+1, dx] is tridiagonal;
                # att[:, w-shifted] += T_dx.T @ m[:, w-shifted].  Vertical
                # padding is implicit in the band clipping, horizontal
                # padding in the PSUM column offsets.
                m32 = small.tile([NQ, QW], bf16, tag="m32")
                nc.scalar.copy(out=m32, in_=msum)
                m_sq = convt.tile([h, w], bf16, tag="msq")
                nc.sync.dma_start(out=m_sq, in_=m32)
                tb = convt.tile([h, h], bf16, tag="tb")
                t_mats = []
                for dx in range(3):
                    T = convt.tile([h, h], bf16, tag=f"T{dx}", name="T")
                    nc.vector.tensor_scalar_mul(
                        out=T, in0=ident, scalar1=kb[:h, 3 + dx : 4 + dx]
                    )
                    nc.vector.tensor_scalar_mul(
                        out=tb, in0=d_up, scalar1=kb[:h, dx : dx + 1]
                    )
                    nc.vector.tensor_add(out=T, in0=T, in1=tb)
                    nc.vector.tensor_scalar_mul(
                        out=tb, in0=d_dn, scalar1=kb[:h, 6 + dx : 7 + dx]
                    )
                    nc.vector.tensor_add(out=T, in0=T, in1=tb)
                    t_mats.append(T)
                attp = pm.tile([h, w], f32, tag="msum", name="attp")
                nc.tensor.matmul(attp, t_mats[1], m_sq, start=True, stop=False)
                nc.tensor.matmul(
                    attp[:, 0 : w - 1],
                    t_mats[2],
                    m_sq[:, 1:w],
                    start=False,
                    stop=False,
                )
                nc.tensor.matmul(
                    attp[:, 1:w],
                    t_mats[0],
                    m_sq[:, 0 : w - 1],
                    start=False,
                    stop=True,
                )
                s_bf = convt.tile([h, w], bf16, tag="sbf")
                nc.scalar.activation(out=s_bf, in_=attp, func=AF.Sigmoid)
                # stage s onto partitions 0/64 (legal matmul base
                # partitions) so the K=1 broadcast matmuls can read it
                # two DMAs so the first half's broadcast matmuls don't
                # wait on the second half's staging
                sr = srp.tile([128, SRW], bf16, tag="sr", name="sr")
                nc.sync.dma_start(out=sr[0:1, :], in_=s_bf[0:64, :])
                nc.sync.dma_start(out=sr[64:65, :], in_=s_bf[64:128, :])
                return sr

            def emit_mults(sr, xres):
                """Broadcast s via K=1 matmuls, stage PSUM->SBUF bf16 on
                ScalarE, multiply x in place on VectorE (2x all-bf16)."""
                for m in range(NM):
                    bp = pb.tile([128, MW], f32, tag="bp", name="bp")
                    for s in range(MW // QW):
                        Q = (MW // QW) * m + s
                        r = 64 * (Q // QPR)
                        nc.tensor.matmul(
                            bp[:, QW * s : QW * (s + 1)],
                            ones_r[r : r + 1, :],
                            sr[r : r + 1, QW * (Q % QPR) : QW * (Q % QPR + 1)],
                            start=True,
                            stop=True,
                        )
                    bps = bsp.tile([128, MW], bf16, tag="bps", name="bps")
                    nc.scalar.copy(out=bps, in_=bp)
                    for hh in range(2):
                        nc.vector.tensor_mul(
                            out=xres[hh][:, MW * m : MW * (m + 1)],
                            in0=xres[hh][:, MW * m : MW * (m + 1)],
                            in1=bps,
                        )

            def emit_stores(b, xres):
                """Stores (bf16) on the two HWDGE rings, concurrent with
                the next sample's loads on the SWDGE queue.  The last
                sample has no loads behind it, so its stores also use the
                idle SWDGE queue as a third ring."""
                if b == bs - 1:
                    # drain phase: nothing left to overlap, so store at
                    # multiply granularity on all three queues to start
                    # the write stream as early as possible
                    engs = [nc.scalar, nc.sync, nc.gpsimd]
                    i = 0
                    for m in range(NM):
                        for hh in range(2):
                            engs[i % 3].dma_start(
                                out=out_d[
                                    b,
                                    128 * hh : 128 * (hh + 1),
                                    MW * m : MW * (m + 1),
                                ],
                                in_=xres[hh][:, MW * m : MW * (m + 1)],
                            )
                            i += 1
                else:
                    for q in range(NCH):
                        for hh in range(2):
                            out_eng = nc.scalar if (q + hh) % 2 == 0 else nc.sync
                            out_eng.dma_start(
                                out=out_d[
                                    b,
                                    128 * hh : 128 * (hh + 1),
                                    CH * q : CH * (q + 1),
                                ],
                                in_=xres[hh][:, CH * q : CH * (q + 1)],
                            )

            for b in range(bs):
                msum, parts, xres = emit_loads(b)
                # chain ops are latency-critical (they gate the whole
                # multiply+store phase); keep the static scheduler from
                # burying them behind ready bulk work
                with tc.high_priority():
                    sr = emit_chain(msum, parts)
                emit_mults(sr, xres)
                emit_stores(b, xres)

    nc.finalize()
    return nc


_NC_CACHE = {}


def _get_nc(key=(BS, C, H, W)):
    if key not in _NC_CACHE:
        _NC_CACHE[key] = build_nc(*key)
    return _NC_CACHE[key]


def kernel(x, w1, b1, w2, b2):
    from concourse.bass_utils import run_bass_kernel_spmd

    x = np.ascontiguousarray(x, dtype=np.float32)
    nc = _get_nc()
    in_maps = []
    for i in range(N_CORES):
        in_maps.append(
            {
                "x": x[i * BS : (i + 1) * BS].reshape(BS, C, HW),
                "w1": np.ascontiguousarray(w1, dtype=np.float32),
                "b1": np.ascontiguousarray(b1, dtype=np.float32),
                "w2": np.ascontiguousarray(w2, dtype=np.float32),
                "b2": np.ascontiguousarray(b2, dtype=np.float32),
            }
        )
    res = run_bass_kernel_spmd(nc, in_maps, list(range(N_CORES)))
    out = np.concatenate(
        [
            np.asarray(r["out"]).astype(np.float32).reshape(BS, C, H, W)
            for r in res.results
        ],
        axis=0,
    )
    return out
